# revision 2
# baseline (speedup 1.0000x reference)
"""Cross-attention (B=2, S=T=2048, H=1024, 16 heads x 64) on 8 trn2 NeuronCores.

Sharding: core c handles batch b = c // 4 and head group hp = c % 4
(4 heads = 256 hidden columns). Data parallel on B, tensor parallel on heads,
row-parallel c_proj with the cross-core reduction done on the host.

Per-core kernel (all matmuls fp32r, full PE rate at N>=256):
  - inputs are fed TRANSPOSED (queryT/key_valueT [H, S]) so every projection
    is PE-natural: kT[d,t] / qT[d,s] via lhsT=W chunk, rhs=xT chunk;
    v[t,d] via lhsT=xT chunk, rhs=Wv chunk.
  - scores computed transposed (scoresT[t,s] = kT.T @ qT) with K=64
    row-tiled head pairs (heads at partitions 0:64 / 64:128 concurrently).
  - exp on ACT (PSUM -> SBUF, float32r out); softmax denominators come for
    free from an augmented V ([v_h | ones], M=65): PV matmul accumulates
    yT (rows 0:64) and sum_t(exp) (row 64) in one PSUM group.
  - normalize: reciprocal of den row, broadcast across partitions via a
    K=1 ones matmul, multiply, cast to f32r via ACT copy.
  - c_proj: outT[o,s] += Wc_h.T @ yTn_h per head (K=64), partial over the
    core's 256 hidden rows; host sums 4 cores per batch and transposes.
"""
import sys

sys.path.insert(0, "/opt/trn_rl_repo")

import numpy as np
from contextlib import ExitStack

import concourse.bass as bass
import concourse.tile as tile
from concourse import bacc, mybir
from concourse.bass import ts
from concourse.bass_utils import run_bass_kernel_spmd

P = 128
S = 2048
T = 2048
H = 1024
DC = 256          # hidden columns per core (4 heads x 64)
NKC = H // P      # 8 contraction chunks
NJ = T // P       # 16 t-chunks
NSG = 4           # s groups of 512
SG = S // NSG
f32 = mybir.dt.float32
f32r = mybir.dt.float32r
Exp = mybir.ActivationFunctionType.Exp

_CACHED = {}


def _build():
    nc = bacc.Bacc("TRN2", target_bir_lowering=False, debug=False)
    qTd = nc.dram_tensor("qTd", [H, S], f32, kind="ExternalInput").ap()
    kvTd = nc.dram_tensor("kvTd", [H, T], f32, kind="ExternalInput").ap()
    wq = nc.dram_tensor("wq", [H, DC], f32, kind="ExternalInput").ap()
    wk = nc.dram_tensor("wk", [H, DC], f32, kind="ExternalInput").ap()
    wv = nc.dram_tensor("wv", [H, DC], f32, kind="ExternalInput").ap()
    wc = nc.dram_tensor("wc", [DC, H], f32, kind="ExternalInput").ap()
    vones = nc.dram_tensor("vones", [P, NJ * 4], f32, kind="ExternalInput").ap()
    outT = nc.dram_tensor("outT", [H, S], f32, kind="ExternalOutput").ap()

    with tile.TileContext(nc) as tc, ExitStack() as ctx:
        wp = ctx.enter_context(tc.tile_pool(name="wp", bufs=1))
        iop = ctx.enter_context(tc.tile_pool(name="iop", bufs=10))
        pers = ctx.enter_context(tc.tile_pool(name="pers", bufs=1))
        qtp = ctx.enter_context(tc.tile_pool(name="qtp", bufs=2))
        expp = ctx.enter_context(tc.tile_pool(name="expp", bufs=3))
        nrm = ctx.enter_context(tc.tile_pool(name="nrm", bufs=2))
        outp = ctx.enter_context(tc.tile_pool(name="outp", bufs=4))
        psum = ctx.enter_context(tc.tile_pool(name="psum", bufs=1, space="PSUM"))

        # --- weights ---
        wq_t = wp.tile([P, NKC, DC], f32r, name="wq_t")
        nc.sync.dma_start(wq_t[:], wq.rearrange("(c p) m -> p c m", p=P).bitcast(f32r))
        wk_t = wp.tile([P, NKC, DC], f32r, name="wk_t")
        nc.sync.dma_start(wk_t[:], wk.rearrange("(c p) m -> p c m", p=P).bitcast(f32r))
        wv_t = wp.tile([P, NKC, DC], f32r, name="wv_t")
        nc.sync.dma_start(wv_t[:], wv.rearrange("(c p) m -> p c m", p=P).bitcast(f32r))
        wc4 = wp.tile([64, 4, H], f32r, name="wc4")
        nc.sync.dma_start(wc4[:], wc.rearrange("(h p) m -> p h m", p=64).bitcast(f32r))
        ones1 = wp.tile([P, 64], f32, name="ones1")
        nc.vector.memset(ones1[:], 1.0)

        kT_t = pers.tile([P, 2, T], f32r, name="kT_t")
        v_t = pers.tile([P, NJ, 4 * 65], f32r, name="v_t")
        nc.sync.dma_start(
            v_t[:].rearrange("p j (h x) -> p j h x", x=65)[:, :, :, 64],
            vones.rearrange("p (j h) -> p j h", j=NJ).bitcast(f32r),
        )

        # --- phase 1: kT and augmented v ---
        np_i = 0  # alternate psum tags "o"/"bc" for projection tiles
        for n in range(4):  # t blocks of 512
            kvb = []
            for c in range(NKC):
                t = iop.tile([P, SG], f32r, tag="io", name=f"kvb{c}")
                nc.sync.dma_start(t[:], kvTd[ts(c, P), ts(n, SG)].bitcast(f32r))
                kvb.append(t)
            for m in range(2):
                pt = psum.tile([P, SG], f32, tag=("o", "bc")[np_i % 2], name="pp")
                np_i += 1
                for c in range(NKC):
                    nc.tensor.matmul(pt[:], wk_t[:, c, ts(m, P)], kvb[c][:],
                                     start=(c == 0), stop=(c == NKC - 1))
                nc.scalar.copy(kT_t[:, m, ts(n, SG)], pt[:])
            for tl in range(4):
                tg = 4 * n + tl
                pt = psum.tile([P, SG], f32, tag=("o", "bc")[np_i % 2], name="pp")
                np_i += 1
                for c in range(NKC):
                    nc.tensor.matmul(pt[:, :DC], kvb[c][:, ts(tl, P)], wv_t[:, c, :],
                                     start=(c == 0), stop=(c == NKC - 1))
                nc.scalar.copy(
                    v_t[:, tg].rearrange("p (h x) -> p h x", x=65)[:, :, 0:64],
                    pt[:, :DC].rearrange("p (h x) -> p h x", x=64),
                )

        # --- phase 2: attention + c_proj per s-group ---
        for sg in range(NSG):
            qb = []
            for c in range(NKC):
                t = iop.tile([P, SG], f32r, tag="io", name=f"qb{c}")
                nc.sync.dma_start(t[:], qTd[ts(c, P), ts(sg, SG)].bitcast(f32r))
                qb.append(t)
            qts = qtp.tile([P, 2, SG], f32r, tag="qts", name="qts")
            for m in range(2):
                pt = psum.tile([P, SG], f32, tag=("o", "bc")[np_i % 2], name="pp")
                np_i += 1
                for c in range(NKC):
                    nc.tensor.matmul(pt[:], wq_t[:, c, ts(m, P)], qb[c][:],
                                     start=(c == 0), stop=(c == NKC - 1))
                nc.scalar.copy(qts[:, m, :], pt[:])

            yts = []
            for p in range(2):
                ya_e = psum.tile([65, SG], f32, tag="ya", bufs=2, name="ya_e")
                ya_o = psum.tile([65, SG], f32, tag="ya", bufs=2, name="ya_o")
                for j in range(NJ):
                    sc = psum.tile([P, 2 * SG], f32, tag="sc", bufs=2, name="sc")
                    nc.tensor.matmul(sc[:, 0:SG], kT_t[0:64, p, ts(j, P)],
                                     qts[0:64, p, :], start=True, stop=True)
                    nc.tensor.matmul(sc[:, SG:2 * SG], kT_t[64:P, p, ts(j, P)],
                                     qts[64:P, p, :], start=True, stop=True)
                    ex = expp.tile([P, 2 * SG], f32r, tag="ex", name="ex")
                    nc.scalar.activation(ex[:], sc[:], Exp)
                    first, last = j == 0, j == NJ - 1
                    h_e, h_o = 2 * p, 2 * p + 1
                    nc.tensor.matmul(ya_e[:], v_t[:, j, h_e * 65:(h_e + 1) * 65],
                                     ex[:, 0:SG], start=first, stop=last)
                    nc.tensor.matmul(ya_o[:], v_t[:, j, h_o * 65:(h_o + 1) * 65],
                                     ex[:, SG:2 * SG], start=first, stop=last)
                for ya_t in (ya_e, ya_o):
                    rsb = nrm.tile([P, SG], f32, tag="rsb", name="rsb")
                    nc.vector.reciprocal(rsb[64:65, :], ya_t[64:65, :])
                    bc = psum.tile([64, SG], f32, tag="bc", name="bc")
                    nc.tensor.matmul(bc[:], ones1[64:65, :], rsb[64:65, :],
                                     start=True, stop=True)
                    rbc = nrm.tile([64, SG], f32, tag="rbc", name="rbc")
                    nc.scalar.copy(rbc[:], bc[:])
                    ytf = nrm.tile([64, SG], f32, tag="ytf", name="ytf")
                    nc.vector.tensor_mul(ytf[:], ya_t[0:64, :], rbc[:])
                    yt = nrm.tile([64, SG], f32r, tag="yt", bufs=6, name="yt")
                    nc.scalar.copy(yt[:], ytf[:])
                    yts.append(yt)
            for oc in range(8):
                pt = psum.tile([P, SG], f32, tag="o", name="opp")
                for h in range(4):
                    nc.tensor.matmul(pt[:], wc4[:, h, ts(oc, P)], yts[h][:],
                                     start=(h == 0), stop=(h == 3))
                ot = outp.tile([P, SG], f32, tag="ot", name="ot")
                nc.vector.tensor_copy(ot[:], pt[:])
                nc.sync.dma_start(outT[ts(oc, P), ts(sg, SG)], ot[:])
    nc.compile()
    return nc


def kernel(query, key_value, Wq, Wkv, Wc):
    query = np.ascontiguousarray(query, np.float32)
    key_value = np.ascontiguousarray(key_value, np.float32)
    Wq = np.asarray(Wq, np.float32)
    Wkv = np.asarray(Wkv, np.float32)
    Wc = np.asarray(Wc, np.float32)
    B = query.shape[0]
    assert query.shape == (2, S, H) and key_value.shape == (2, T, H)

    if "nc" not in _CACHED:
        _CACHED["nc"] = _build()
    nc = _CACHED["nc"]

    scale = (H // 16) ** -0.5  # head_dim ** -0.5
    vones = np.ones((P, NJ * 4), np.float32)
    in_maps = []
    for c in range(8):
        b, hp = divmod(c, 4)
        sl = slice(hp * DC, (hp + 1) * DC)
        in_maps.append({
            "qTd": np.ascontiguousarray(query[b].T),
            "kvTd": np.ascontiguousarray(key_value[b].T),
            "wq": np.ascontiguousarray(Wq[:, sl]) * np.float32(scale),
            "wk": np.ascontiguousarray(Wkv[:, sl]),
            "wv": np.ascontiguousarray(Wkv[:, H + hp * DC:H + (hp + 1) * DC]),
            "wc": np.ascontiguousarray(Wc[sl, :]),
            "vones": vones,
        })

    res = run_bass_kernel_spmd(nc, in_maps, core_ids=list(range(8)))
    outs = [r["outT"] for r in res.results]
    out = np.empty((B, S, H), np.float32)
    for b in range(B):
        acc = outs[4 * b] + outs[4 * b + 1] + outs[4 * b + 2] + outs[4 * b + 3]
        out[b] = acc.T
    return out
